# revision 59
# baseline (speedup 1.0000x reference)
"""GraphSage 3-layer GNN on 8 Trainium2 NeuronCores.

Strategy: shard nodes (rows of A) across the 8 cores. A is a 0/1
adjacency (plus exact 1.0 self-loops) => exact in fp8_e4m3, so the
per-core A^T shard (12288x1536 = 18.9 MB fp8) is streamed from DRAM
ONCE and kept RESIDENT in SBUF; layers 2-3 re-read it with zero HBM
traffic, in fp8 DoubleRow mode. The shard is stored partition-major in
COLUMN-THIRDS matching layer 1's three 512-node m-tiles (4KB-contiguous
descriptors, all 36 stream DMAs alone on the sync queue so DMA-ring
backpressure never blocks other work): m-tile t's matmuls finish right
after third t lands, so its tail + AllGather overlap the rest of the
stream and absorb cross-core launch skew.

Inter-layer h exchange is a PIPELINED per-m-tile AllGather; the next
layer consumes k-pairs grouped by source AG so each third unblocks as
its gather lands. The global-sum-pool uses a tiny AllGather + on-core
reduction (cheaper than AllReduce) with the per-m-tile reduction fused
into the tanh's accum_out.

ACT table thrash (sqrt vs tanh live in different 1.3us-load tables) is
hidden by dummy 1-element activations data-anchored behind the real ops
so table loads run under DVE work. Each tail's tanh-gated transposes +
export are DEFERRED past the next m-tile's matmuls: the PE wait-queue
holds only 4 blocked instructions before head-of-line blocking. Warm
dummy-matmul chains keep the PE HAM clock at 2.4 GHz across stream
ramp-up and AllGather gaps.

Numerics: stationary x/h quantized to fp8_e4m3 (RNE) only for the
mean-aggregation matmul; the concatenated self-features stay f32.
~5e-3 relative error vs the 2e-2 gate (A itself is exact in fp8).
"""

import os
import sys
import types

import numpy as np

# ---------------------------------------------------------------- ntff hook
# The image lacks antenv.axon_hooks; inject it so trace=True (profiling,
# enabled via BASS_TRACE=1 by test.py) can capture NTFF under axon.
def _install_ntff_hook():
    if "antenv.axon_hooks" in sys.modules:
        return
    try:
        import antenv
        mod = types.ModuleType("antenv.axon_hooks")
        _hook = [None]
        mod.set_axon_ntff_profile_hook = lambda h: _hook.__setitem__(0, h)
        mod.get_axon_ntff_profile_hook = lambda: _hook[0]
        sys.modules["antenv.axon_hooks"] = mod
        antenv.axon_hooks = mod
        from trn_agent_boot.trn_boot import _ntff_profile_via_ctypes
        so = "/opt/axon/libaxon_pjrt.so"
        if os.path.exists(so):
            mod.set_axon_ntff_profile_hook(_ntff_profile_via_ctypes(so))
    except Exception:
        pass


_install_ntff_hook()

import ml_dtypes  # noqa: E402
import concourse.bass as bass  # noqa: E402
import concourse.bacc as bacc  # noqa: E402
import concourse.tile as tile  # noqa: E402
import concourse.mybir as mybir  # noqa: E402
from concourse.bass_utils import run_bass_kernel_spmd  # noqa: E402

# ------------------------------------------------------------------ geometry
N = 12000          # real nodes
F = 128            # input feature dim
H = 32             # hidden dim
NC = 8             # cores
NP = 12288         # padded nodes  (= 96*128 = 8*1536)
SH = NP // NC      # 1536 rows per core
KC = NP // 128     # 96 contraction chunks
KP = KC // 2       # 48 DoubleRow k-pairs
MT = [(0, 512), (512, 512), (1024, 512)]   # m-tiles within the shard
NJ = SH // 128     # 12 transpose subtiles
TOL = 1e-6

WARM = int(os.environ.get("KWARM", "12"))  # warm-chain matmuls per boundary

F32 = mybir.dt.float32
F32R = mybir.dt.float32r
BF16 = mybir.dt.bfloat16
FP8 = mybir.dt.float8e4
NP_FP8 = ml_dtypes.float8_e4m3
NP_BF16 = ml_dtypes.bfloat16
DR = mybir.MatmulPerfMode.DoubleRow
TANH = mybir.ActivationFunctionType.Tanh
SQRT = mybir.ActivationFunctionType.Sqrt

LAST_EXEC_NS = None
_CACHE = {}


# Contraction-chunk permutation. Each layer boundary runs THREE
# pipelined AllGathers (one per m-tile of h). Chunk k' = 32*mi + 4*c + j
# holds global node chunk k = 12*c + 4*mi + j (core c, m-tile mi,
# subchunk j), so AG #mi fills the CONTIGUOUS h_stat slice
# [32*mi, 32*mi+32) in one 3D DMA, and consuming k-pairs in order gives
# AG-group boundaries at pairs 16 and 32.
CHUNK_PERM = [12 * ((kp % 32) // 4) + 4 * (kp // 32) + (kp % 4)
              for kp in range(KC)]
assert sorted(CHUNK_PERM) == list(range(KC))


# ------------------------------------------------------------------- builder
def _build():
    nc = bacc.Bacc("TRN2", target_bir_lowering=False, debug=False,
                   num_devices=NC)

    at_d = nc.dram_tensor("at", [128, 3 * KC * 512], FP8,
                          kind="ExternalInput")
    xs_d = nc.dram_tensor("xs", [128, KC * F], FP8, kind="ExternalInput")
    xt_d = nc.dram_tensor("xt", [F, SH], BF16, kind="ExternalInput")
    rc_d = nc.dram_tensor("rc", [F, SH], BF16, kind="ExternalInput")
    w1t_d = nc.dram_tensor("w1t", [F, H], BF16, kind="ExternalInput")
    w1b_d = nc.dram_tensor("w1b", [F, H], F32R, kind="ExternalInput")
    w2t_d = nc.dram_tensor("w2t", [H, H], F32R, kind="ExternalInput")
    w2b_d = nc.dram_tensor("w2b", [H, H], F32R, kind="ExternalInput")
    w3t_d = nc.dram_tensor("w3t", [H, H], F32R, kind="ExternalInput")
    w3b_d = nc.dram_tensor("w3b", [H, H], F32R, kind="ExternalInput")
    wf1p_d = nc.dram_tensor("wf1p", [128, 2 * H], F32, kind="ExternalInput")
    wf2_d = nc.dram_tensor("wf2", [2 * H, 1], F32, kind="ExternalInput")
    b1_d = nc.dram_tensor("b1", [H, 1], F32, kind="ExternalInput")
    b2_d = nc.dram_tensor("b2", [H, 1], F32, kind="ExternalInput")
    b3_d = nc.dram_tensor("b3", [H, 1], F32, kind="ExternalInput")
    bf1_d = nc.dram_tensor("bf1", [2 * H, 1], F32, kind="ExternalInput")
    bf2_d = nc.dram_tensor("bf2", [1, 1], F32, kind="ExternalInput")
    i32_d = nc.dram_tensor("i32", [32, 32], F32R, kind="ExternalInput")
    ones_d = nc.dram_tensor("ones", [H, H], F32R, kind="ExternalInput")
    out_d = nc.dram_tensor("out", [1, 1], F32, kind="ExternalOutput")

    # per-(layer, m-tile) pipelined AllGather staging: each core
    # contributes its 512-node h slice in node-major fp8; gather output
    # is [NC*512, H] ordered (core, subchunk j, partition p).
    ag_in = [[nc.dram_tensor(f"ag_in{l}_{m}", [512, H], FP8)
              for m in range(3)] for l in range(2)]
    ag_out = [[nc.dram_tensor(f"ag_out{l}_{m}", [NC * 512, H], FP8,
                              addr_space="Shared")
               for m in range(3)] for l in range(2)]
    # global-sum-pool exchange: AllGather of each core's [H,1] partial
    agp_in = nc.dram_tensor("agp_in", [H, 1], F32)
    agp_out = nc.dram_tensor("agp_out", [NC * H, 1], F32,
                             addr_space="Shared")
    rg = [list(range(NC))]

    with tile.TileContext(nc) as tc:
        with (
            tc.tile_pool(name="const", bufs=1) as constp,
            tc.tile_pool(name="atres", bufs=1) as atresp,
            tc.tile_pool(name="xstat", bufs=1) as xstatp,
            tc.tile_pool(name="hstat", bufs=2) as hstatp,
            tc.tile_pool(name="hT", bufs=2) as hTp,
            tc.tile_pool(name="hnat", bufs=2) as hnatp,
            tc.tile_pool(name="ep", bufs=2) as ep,
            tc.tile_pool(name="znp", bufs=1) as znp,
            tc.tile_pool(name="agg_ps", bufs=3, space=bass.MemorySpace.PSUM) as agg_ps,
            tc.tile_pool(name="z_ps", bufs=2, space=bass.MemorySpace.PSUM) as z_ps,
            tc.tile_pool(name="bc_ps", bufs=1, space=bass.MemorySpace.PSUM) as bc_ps,
            tc.tile_pool(name="t_ps", bufs=2, space=bass.MemorySpace.PSUM) as t_ps,
        ):
            # x-stationary first on gpsimd: host stores xs partition-major
            # ([128, KC*F]) so each load is one contiguous 6KB descriptor
            # per partition — two issues total.
            xs = xstatp.tile([128, KC, F], FP8)
            xs_r = xs_d.ap().rearrange("p (k f) -> p k f", f=F)
            nc.gpsimd.dma_start(xs[:, 0:48, :], xs_r[:, 0:48, :])
            nc.gpsimd.dma_start(xs[:, 48:96, :], xs_r[:, 48:96, :])

            # resident A^T shard, streamed in COLUMN-THIRDS matching the
            # three m-tiles: L1's m-tile 0 finishes right after the first
            # third lands, so its tail + AllGather overlap the remaining
            # two thirds of the stream (and absorb cross-core launch
            # skew). Host stores the shard partition-major third-major so
            # every 8-chunk DMA is 128 contiguous 4KB descriptors. All 36
            # stream DMAs ride the sync queue ALONE — DMA-ring
            # backpressure on a shared queue would block tail work.
            att = atresp.tile([128, 3, KC, 512], FP8)
            at_r = at_d.ap().rearrange("p (t k s) -> p t k s", t=3, s=512)
            for t in range(3):
                for g in range(12):
                    nc.sync.dma_start(
                        att[:, t, 8 * g:8 * g + 8, :],
                        at_r[:, t, 8 * g:8 * g + 8, :])

            # ---- constants
            def cload(dram, shape, dt=F32):
                t = constp.tile(shape, dt, tag=dram.name)
                nc.gpsimd.dma_start(t[:], dram[:, :])
                return t

            w1t = cload(w1t_d, [F, H], BF16)
            w1b = cload(w1b_d, [F, H], F32R)
            w2t = cload(w2t_d, [H, H], F32R)
            w2b = cload(w2b_d, [H, H], F32R)
            w3t = cload(w3t_d, [H, H], F32R)
            w3b = cload(w3b_d, [H, H], F32R)
            wf1p = cload(wf1p_d, [128, 2 * H])
            wf2 = cload(wf2_d, [2 * H, 1])
            b1 = cload(b1_d, [H, 1])
            b2 = cload(b2_d, [H, 1])
            b3 = cload(b3_d, [H, 1])
            bf1 = cload(bf1_d, [2 * H, 1])
            bf2 = cload(bf2_d, [1, 1])
            i32 = cload(i32_d, [32, 32], F32R)
            ones_m = cload(ones_d, [H, H], F32R)
            xt = cload(xt_d, [F, SH], BF16)
            rc = cload(rc_d, [F, SH], BF16)

            # ACT table prefetch targets: 1-elem dummy activations force
            # the ~1.3us table load to happen while other engines work.
            dmy = constp.tile([1, 1], F32, tag="dmy")

            def prefetch(func, _name):
                nc.scalar.activation(dmy[:, :], bf2[:1, :1], func)

            # preload the SQRT table during the L1 stream
            prefetch(SQRT, "pre_sqrt_init")

            def big_matmul(pagg, h_stat, m0, mw, j, start, stop):
                nc.tensor.matmul(
                    pagg[:, :mw], h_stat[:, 2 * j:2 * j + 2, :],
                    att[:, m0 // 512, 2 * j:2 * j + 2, :mw],
                    start=start, stop=stop, perf_mode=DR)

            def tail(li, fl, mi, m0, mw, pagg, hT_in, wtop, wbot, b,
                     hTn, hnat, red=None, last=False):
                """per-m-tile epilogue: mean-scale, dense, l2norm, tanh,
                then transpose the fp8 node-major h; m-tile 0 exports and
                triggers AG_a, m-tile 2 exports m-tiles 1+2 as AG_b.
                red: (tile, col) partial sum-pool column (fused into the
                tanh's accum_out)."""
                # mean-scale stays on DVE (gpsimd has no PSUM port)
                aggs = ep.tile([F, 512], F32R, tag="aggs")
                nc.vector.tensor_mul(
                    aggs[:fl, :mw], pagg[:fl, :mw], rc[:fl, m0:m0 + mw])
                pz = z_ps.tile([H, 512], F32, tag="pz")
                nc.tensor.matmul(pz[:, :mw], wtop[:, :], hT_in[:, m0:m0 + mw],
                                 start=True, stop=False)
                nc.tensor.matmul(pz[:, :mw], wbot[:, :], aggs[:fl, :mw],
                                 start=False, stop=True)
                zb = ep.tile([H, 512], F32, tag="zb")
                nc.vector.tensor_scalar_add(zb[:, :mw], pz[:, :mw], b[:])
                # row l2-norm over features (partition dim): sumsq via
                # ones-matmul broadcast back to H partitions.
                sq = ep.tile([H, 512], F32R, tag="sqzn")
                nc.vector.tensor_mul(sq[:, :mw], zb[:, :mw], zb[:, :mw])
                pbc = bc_ps.tile([H, 512], F32, tag="pbc")
                nc.tensor.matmul(pbc[:, :mw], ones_m[:, :], sq[:, :mw],
                                 start=True, stop=True)
                ssb = ep.tile([H, 512], F32, tag="ssb")
                nc.vector.tensor_scalar_max(ssb[:, :mw], pbc[:, :mw], 1e-12)
                srt = ep.tile([H, 512], F32, tag="sqzn")
                nc.scalar.sqrt(srt[:, :mw], ssb[:, :mw])
                if red is not None and mi > 0:
                    # pool layer (L3), m-tiles 1-2: DEFER the tanh so
                    # sqrt1/sqrt2 share SQRT table residency and the two
                    # tanhs share one TANH load at the layer end.
                    # recip+mul run eagerly on DVE.
                    rn = ep.tile([H, 512], F32, tag="rn")
                    nc.vector.reciprocal_approx_fast(rn[:, :mw],
                                                     srt[:, :mw])
                    znd = znp.tile([H, 512], F32, tag=f"znd{mi}")
                    nc.vector.tensor_mul(znd[:, :mw], zb[:, :mw],
                                         rn[:, :mw])
                    rtile, rcol = red

                    def fin():
                        nc.scalar.activation(
                            hTn[:, m0:m0 + mw], znd[:, :mw], TANH,
                            accum_out=rtile[:, rcol:rcol + 1])
                    return (fin, znd)
                # swap the table to TANH under the DVE recip+mul. The
                # dummy READS the sqrt's output so the scheduler cannot
                # hoist it ahead of the real sqrt.
                nc.scalar.activation(dmy[:, :], srt[0:1, 0:1], TANH)
                rn = ep.tile([H, 512], F32, tag="rn")
                nc.vector.reciprocal_approx_fast(rn[:, :mw], srt[:, :mw])
                zn = ep.tile([H, 512], F32, tag="sqzn")
                nc.vector.tensor_mul(zn[:, :mw], zb[:, :mw], rn[:, :mw])
                if red is not None:
                    rtile, rcol = red
                    nc.scalar.activation(hTn[:, m0:m0 + mw], zn[:, :mw],
                                         TANH,
                                         accum_out=rtile[:, rcol:rcol + 1])
                else:
                    nc.scalar.activation(hTn[:, m0:m0 + mw], zn[:, :mw],
                                         TANH)
                if not last:
                    # re-arm the SQRT table for the next tail, anchored
                    # behind the real tanh via a read of its output.
                    nc.scalar.activation(dmy[:, :], hTn[0:1, m0:m0 + 1],
                                         SQRT)
                if hnat is None:
                    return None

                def export(h_stat):
                    """transpose to node-major, export, AllGather, and
                    reload the gathered chunk region. DEFERRED by the
                    caller until after the next m-tile's matmuls are
                    emitted: the PE wait-queue holds only 4 blocked
                    instructions, so emitting the tanh-gated transposes
                    before ready matmuls head-of-line-blocks the PE."""
                    js = m0 // 128
                    for j in range(js, js + 4):
                        pt = t_ps.tile([128, H], F32R, tag="pt")
                        nc.tensor.transpose(
                            pt[:, :], hTn[:, j * 128:(j + 1) * 128],
                            i32[:, :])
                        nc.vector.tensor_copy(hnat[:, j, :], pt[:, :])
                    agr = ag_in[li][mi].ap().rearrange(
                        "(j p) f -> p j f", p=128)
                    nc.scalar.dma_start(agr[:, :, :], hnat[:, js:js + 4, :])
                    nc.gpsimd.collective_compute(
                        "AllGather", mybir.AluOpType.bypass,
                        replica_groups=rg,
                        ins=[ag_in[li][mi].ap().opt()],
                        outs=[ag_out[li][mi].ap().opt()])
                    if h_stat is not None:
                        load_hstat(li, h_stat, mi)
                return export

            def warm_chain(tag, n):
                """dummy matmuls to keep the PE HAM-warm across a gap."""
                if n <= 0:
                    return
                pw = bc_ps.tile([H, 512], F32, tag="pbc", name=f"warm{tag}")
                for d in range(n):
                    nc.tensor.matmul(pw[:, :], xs[:, 0, 0:H], xs[:, 0:4, :],
                                     start=(d == 0), stop=(d == n - 1))

            def load_hstat(li, h_stat, mi):
                """pull AG #mi's output into the stationary h tile: with
                the CHUNK_PERM layout these are chunks [32mi, 32mi+32)."""
                agor = ag_out[li][mi].ap().rearrange(
                    "(k p) f -> p k f", p=128)
                nc.sync.dma_start(h_stat[:, 32 * mi:32 * mi + 32, :],
                                  agor)

            # ----------------- layer 1: column-third-outer, k-inner;
            # m-tile t's tail + AllGather overlap thirds t+1.. of the
            # stream.
            hT1 = hTp.tile([H, SH], F32R, tag="hTn", name="hTn0")
            hnat0 = hnatp.tile([128, NJ, H], FP8, tag="hnat", name="hnat0")
            hs1 = hstatp.tile([128, KC, H], FP8, tag="hstat", name="hstat0")
            # get the PE HAM-warm (2.4 GHz) before/while the stream starts:
            # a cold PE (1.2 GHz) paces L1 slower than the DMA stream.
            warm_chain("pre", 18)
            pend = None
            for t, (m0, mw) in enumerate(MT):
                pagg = agg_ps.tile([F, 512], F32, tag="pagg",
                                   name=f"pagg0_{t}")
                for j in range(KP):
                    big_matmul(pagg, xs, m0, mw, j,
                               start=(j == 0), stop=(j == KP - 1))
                    if t == 0 and 0 < j < 8:
                        # pad early DMA-wait gaps so HAM's activity window
                        # sees a busy PE and un-throttles to full clock
                        warm_chain(f"pad{j}", 3)
                    if j == 3 and pend is not None:
                        # a 4-pair prefix of ready matmuls sits ahead of
                        # the previous tail's 4 tanh-gated transposes, so
                        # they fill the PE wait-queue without blocking
                        pend(hs1)
                        pend = None
                pend = tail(0, F, t, m0, mw, pagg, xt, w1t, w1b, b1,
                            hT1, hnat0)
            # L2 m-tile-0 HEAD START: its first 16 pairs touch only the
            # AG_a chunk region (landed long ago), so they fill the PE
            # idle window during tail2's chain + AG_c instead of stalling.
            pagg_h = agg_ps.tile([H, 512], F32, tag="pagg", name="pagg1_0")
            for j in range(16):
                big_matmul(pagg_h, hs1, 0, 512, j,
                           start=(j == 0), stop=False)
            warm_chain("sp0", 4)
            pend(hs1)
            warm_chain("b0", 2 * WARM)

            # ----------------- layers 2-3: m-outer, A + h resident in SBUF
            def layer23(li, hs, hT_in, wtop, wbot, b, hnat, pool=None,
                        hs_next=None, head=None):
                hTn = hTp.tile([H, SH], F32R, tag="hTn", name=f"hTn{li}")
                pend = None
                fins = []
                for mi, (m0, mw) in enumerate(MT):
                    if mi == 0 and head is not None:
                        pagg, j0 = head, 16
                    else:
                        pagg = agg_ps.tile([H, 512], F32, tag="pagg",
                                           name=f"pagg{li}_{mi}")
                        j0 = 0
                    for j in range(j0, KP):
                        big_matmul(pagg, hs, m0, mw, j,
                                   start=(j == 0), stop=(j == KP - 1))
                        if j == 3 and pend is not None:
                            pend(hs_next)
                            pend = None
                    if mi == 2 and hnat is not None:
                        warm_chain(f"l{li}", WARM)
                    r = tail(li, H, mi, m0, mw, pagg, hT_in, wtop,
                             wbot, b, hTn, hnat,
                             red=(pool, mi) if pool is not None else None,
                             last=(pool is not None and mi == 2))
                    if pool is not None:
                        if r is not None:
                            fins.append(r)
                    else:
                        pend = r
                if pend is not None:
                    warm_chain(f"sp{li}", 4)
                    pend(hs_next)
                if fins:
                    # one TANH table load, anchored behind the last
                    # m-tile's recip/mul chain, then the deferred tanhs
                    nc.scalar.activation(dmy[:, :], fins[-1][1][0:1, 0:1],
                                         TANH)
                    for fin, _ in fins:
                        fin()
                return hTn

            hnat1 = hnatp.tile([128, NJ, H], FP8, tag="hnat", name="hnat1")
            hs2 = hstatp.tile([128, KC, H], FP8, tag="hstat", name="hstat1")
            hT2 = layer23(1, hs1, hT1, w2t, w2b, b2, hnat1, hs_next=hs2,
                          head=pagg_h)
            warm_chain("b1", 2 * WARM)
            pool4 = ep.tile([H, 4], F32, tag="pT")
            hT3 = layer23(2, hs2, hT2, w3t, w3b, b3, None, pool=pool4)

            # combine the per-m-tile pool partials (padded nodes are 0),
            # AllGather the [H,1] partials, and reduce 8 blocks on-core:
            # [256,1] -> [128,2] -> DVE free-sum -> [128,1] -> i128 matmul.
            pT = ep.tile([H, 1], F32, tag="pS")
            nc.vector.reduce_sum(pT[:, :], pool4[:, 0:3],
                                 axis=mybir.AxisListType.X)
            nc.scalar.dma_start(agp_in[:, :], pT[:])
            nc.gpsimd.collective_compute(
                "AllGather", mybir.AluOpType.bypass, replica_groups=rg,
                ins=[agp_in.ap().opt()], outs=[agp_out.ap().opt()])
            pG = ep.tile([128, 2], F32, tag="pG")
            nc.gpsimd.dma_start(
                pG[:, :], agp_out.ap().rearrange("(a p) c -> p (a c)", a=2))
            pH = ep.tile([128, 1], F32, tag="pH")
            nc.vector.reduce_sum(pH[:, :], pG[:, :],
                                 axis=mybir.AxisListType.X)
            # final MLP (redundant on every core); wf1p = tile(Wf1, (4,1))
            # host-folds the 4-block pool reduction into the dense layer
            pq = z_ps.tile([2 * H, 1], F32, tag="pz")
            nc.tensor.matmul(pq[:, :], wf1p[:, :], pH[:, :], start=True,
                             stop=True)
            q = ep.tile([2 * H, 1], F32, tag="q")
            nc.scalar.activation(q[:, :], pq[:, :], TANH, bias=bf1[:])
            po = z_ps.tile([1, 1], F32, tag="pz")
            nc.tensor.matmul(po[:, :], wf2[:, :], q[:, :], start=True,
                             stop=True)
            ob = ep.tile([1, 1], F32, tag="ob")
            nc.vector.tensor_scalar_add(ob[:, :], po[:, :], bf2[:])
            nc.gpsimd.dma_start(out_d[:, :], ob[:])

    nc.compile()
    return nc


# ---------------------------------------------------------------- host prep
def _prep(inputs):
    x = np.asarray(inputs["x"], np.float32)
    a = np.asarray(inputs["a"], np.float32)
    diag = np.diagonal(a).copy()
    add = (np.abs(diag) < TOL).astype(np.float32)
    deg = a.sum(axis=1) + add          # row sums of a_hat
    recip = np.ones(NP, np.float32)
    recip[:N] = 1.0 / deg

    # row-block permutation implementing CHUNK_PERM (see top of file)
    row_perm = (np.asarray(CHUNK_PERM)[:, None] * 128
                + np.arange(128)[None, :]).reshape(-1)

    x_pad = np.zeros((NP, F), np.float32)
    x_pad[:N] = x
    # partition-major xs: [128, KC*F], row p = concat_k x[chunk k, p]
    xs = (x_pad[row_perm].reshape(KC, 128, F).transpose(1, 0, 2)
          .reshape(128, KC * F).astype(NP_FP8))

    w1 = np.asarray(inputs["W1"], np.float32)
    common = {
        "xs": xs,
        "w1t": w1[:F].astype(NP_BF16), "w1b": w1[F:].copy(),
        "w2t": np.asarray(inputs["W2"], np.float32)[:H].copy(),
        "w2b": np.asarray(inputs["W2"], np.float32)[H:].copy(),
        "w3t": np.asarray(inputs["W3"], np.float32)[:H].copy(),
        "w3b": np.asarray(inputs["W3"], np.float32)[H:].copy(),
        "wf1p": np.tile(np.asarray(inputs["Wf1"], np.float32), (4, 1)),
        "wf2": np.asarray(inputs["Wf2"], np.float32),
        "b1": np.asarray(inputs["b1"], np.float32).reshape(H, 1),
        "b2": np.asarray(inputs["b2"], np.float32).reshape(H, 1),
        "b3": np.asarray(inputs["b3"], np.float32).reshape(H, 1),
        "bf1": np.asarray(inputs["bf1"], np.float32).reshape(2 * H, 1),
        "bf2": np.asarray(inputs["bf2"], np.float32).reshape(1, 1),
        "i32": np.eye(32, dtype=np.float32),
        "ones": np.ones((H, H), dtype=np.float32),
    }

    in_maps = []
    for c in range(NC):
        r0 = c * SH
        r1 = min((c + 1) * SH, N)
        nrow = max(r1 - r0, 0)
        at = np.zeros((NP, SH), NP_FP8)
        if nrow > 0:
            blk = a[r0:r1].T.astype(NP_FP8)         # [N(12000), nrow]
            at[:N, :nrow] = blk
            # self-loops on approximately-zero diagonal entries
            idx = np.arange(nrow)
            gi = r0 + idx
            sel = add[gi] > 0
            at[gi[sel], idx[sel]] = np.asarray(
                a[gi[sel], gi[sel]] + 1.0, NP_FP8)
        # partition-major third-major: row p = concat over (third t,
        # chunk k') of at[128k'+p, 512t:512t+512]
        at = (at[row_perm].reshape(KC, 128, 3, 512)
              .transpose(1, 2, 0, 3).reshape(128, 3 * KC * 512))
        xt = np.zeros((F, SH), NP_BF16)
        if nrow > 0:
            xt[:, :nrow] = x[r0:r1].T.astype(NP_BF16)
        rcb = np.broadcast_to(recip[r0:r0 + SH].astype(NP_BF16),
                              (F, SH)).copy()
        m = dict(common)
        m.update({"at": at, "xt": xt, "rc": rcb})
        in_maps.append(m)
    return in_maps


# -------------------------------------------------------------------- kernel
def kernel(**inputs):
    global LAST_EXEC_NS
    if "nc" not in _CACHE:
        _CACHE["nc"] = _build()
    nc = _CACHE["nc"]
    in_maps = _prep(inputs)
    res = run_bass_kernel_spmd(nc, in_maps, core_ids=list(range(NC)))
    LAST_EXEC_NS = res.exec_time_ns
    return np.asarray(res.results[0]["out"], np.float32).reshape(1, 1)


# revision 60
# speedup vs baseline: 1.0051x; 1.0051x over previous
"""GraphSage 3-layer GNN on 8 Trainium2 NeuronCores.

Strategy: shard nodes (rows of A) across the 8 cores. A is a 0/1
adjacency (plus exact 1.0 self-loops) => exact in fp8_e4m3, so the
per-core A^T shard (12288x1536 = 18.9 MB fp8) is streamed from DRAM
ONCE and kept RESIDENT in SBUF; layers 2-3 re-read it with zero HBM
traffic, in fp8 DoubleRow mode. The shard is stored partition-major in
COLUMN-THIRDS matching layer 1's three 512-node m-tiles (4KB-contiguous
descriptors, all 36 stream DMAs alone on the sync queue so DMA-ring
backpressure never blocks other work): m-tile t's matmuls finish right
after third t lands, so its tail + AllGather overlap the rest of the
stream and absorb cross-core launch skew.

Inter-layer h exchange is a PIPELINED per-m-tile AllGather; the next
layer consumes k-pairs grouped by source AG so each third unblocks as
its gather lands. The global-sum-pool uses a tiny AllGather + on-core
reduction (cheaper than AllReduce) with the per-m-tile reduction fused
into the tanh's accum_out.

ACT table thrash (sqrt vs tanh live in different 1.3us-load tables) is
hidden by dummy 1-element activations data-anchored behind the real ops
so table loads run under DVE work. Each tail's tanh-gated transposes +
export are DEFERRED past the next m-tile's matmuls: the PE wait-queue
holds only 4 blocked instructions before head-of-line blocking. Warm
dummy-matmul chains keep the PE HAM clock at 2.4 GHz across stream
ramp-up and AllGather gaps.

Numerics: stationary x/h quantized to fp8_e4m3 (RNE) only for the
mean-aggregation matmul; the concatenated self-features stay f32.
~5e-3 relative error vs the 2e-2 gate (A itself is exact in fp8).
"""

import os
import sys
import types

import numpy as np

# ---------------------------------------------------------------- ntff hook
# The image lacks antenv.axon_hooks; inject it so trace=True (profiling,
# enabled via BASS_TRACE=1 by test.py) can capture NTFF under axon.
def _install_ntff_hook():
    if "antenv.axon_hooks" in sys.modules:
        return
    try:
        import antenv
        mod = types.ModuleType("antenv.axon_hooks")
        _hook = [None]
        mod.set_axon_ntff_profile_hook = lambda h: _hook.__setitem__(0, h)
        mod.get_axon_ntff_profile_hook = lambda: _hook[0]
        sys.modules["antenv.axon_hooks"] = mod
        antenv.axon_hooks = mod
        from trn_agent_boot.trn_boot import _ntff_profile_via_ctypes
        so = "/opt/axon/libaxon_pjrt.so"
        if os.path.exists(so):
            mod.set_axon_ntff_profile_hook(_ntff_profile_via_ctypes(so))
    except Exception:
        pass


_install_ntff_hook()

import ml_dtypes  # noqa: E402
import concourse.bass as bass  # noqa: E402
import concourse.bacc as bacc  # noqa: E402
import concourse.tile as tile  # noqa: E402
import concourse.mybir as mybir  # noqa: E402
from concourse.bass_utils import run_bass_kernel_spmd  # noqa: E402

# ------------------------------------------------------------------ geometry
N = 12000          # real nodes
F = 128            # input feature dim
H = 32             # hidden dim
NC = 8             # cores
NP = 12288         # padded nodes  (= 96*128 = 8*1536)
SH = NP // NC      # 1536 rows per core
KC = NP // 128     # 96 contraction chunks
KP = KC // 2       # 48 DoubleRow k-pairs
MT = [(0, 512), (512, 512), (1024, 512)]   # m-tiles within the shard
NJ = SH // 128     # 12 transpose subtiles
TOL = 1e-6

WARM = int(os.environ.get("KWARM", "12"))  # warm-chain matmuls per boundary

F32 = mybir.dt.float32
F32R = mybir.dt.float32r
BF16 = mybir.dt.bfloat16
FP8 = mybir.dt.float8e4
NP_FP8 = ml_dtypes.float8_e4m3
NP_BF16 = ml_dtypes.bfloat16
DR = mybir.MatmulPerfMode.DoubleRow
TANH = mybir.ActivationFunctionType.Tanh
SQRT = mybir.ActivationFunctionType.Sqrt

LAST_EXEC_NS = None
_CACHE = {}


# Contraction-chunk permutation. Each layer boundary runs THREE
# pipelined AllGathers (one per m-tile of h). Chunk k' = 32*mi + 4*c + j
# holds global node chunk k = 12*c + 4*mi + j (core c, m-tile mi,
# subchunk j), so AG #mi fills the CONTIGUOUS h_stat slice
# [32*mi, 32*mi+32) in one 3D DMA, and consuming k-pairs in order gives
# AG-group boundaries at pairs 16 and 32.
CHUNK_PERM = [12 * ((kp % 32) // 4) + 4 * (kp // 32) + (kp % 4)
              for kp in range(KC)]
assert sorted(CHUNK_PERM) == list(range(KC))


# ------------------------------------------------------------------- builder
def _build():
    nc = bacc.Bacc("TRN2", target_bir_lowering=False, debug=False,
                   num_devices=NC)

    at_d = nc.dram_tensor("at", [128, 3 * KC * 512], FP8,
                          kind="ExternalInput")
    xs_d = nc.dram_tensor("xs", [128, KC * F], FP8, kind="ExternalInput")
    xt_d = nc.dram_tensor("xt", [F, SH], BF16, kind="ExternalInput")
    rc_d = nc.dram_tensor("rc", [F, SH], BF16, kind="ExternalInput")
    w1t_d = nc.dram_tensor("w1t", [F, H], BF16, kind="ExternalInput")
    w1b_d = nc.dram_tensor("w1b", [F, H], F32R, kind="ExternalInput")
    w2t_d = nc.dram_tensor("w2t", [H, H], F32R, kind="ExternalInput")
    w2b_d = nc.dram_tensor("w2b", [H, H], F32R, kind="ExternalInput")
    w3t_d = nc.dram_tensor("w3t", [H, H], F32R, kind="ExternalInput")
    w3b_d = nc.dram_tensor("w3b", [H, H], F32R, kind="ExternalInput")
    wf1p_d = nc.dram_tensor("wf1p", [128, 2 * H], F32, kind="ExternalInput")
    wf2_d = nc.dram_tensor("wf2", [2 * H, 1], F32, kind="ExternalInput")
    b1_d = nc.dram_tensor("b1", [H, 1], F32, kind="ExternalInput")
    b2_d = nc.dram_tensor("b2", [H, 1], F32, kind="ExternalInput")
    b3_d = nc.dram_tensor("b3", [H, 1], F32, kind="ExternalInput")
    bf1_d = nc.dram_tensor("bf1", [2 * H, 1], F32, kind="ExternalInput")
    bf2_d = nc.dram_tensor("bf2", [1, 1], F32, kind="ExternalInput")
    i32_d = nc.dram_tensor("i32", [32, 32], F32R, kind="ExternalInput")
    ones_d = nc.dram_tensor("ones", [H, H], F32R, kind="ExternalInput")
    out_d = nc.dram_tensor("out", [1, 1], F32, kind="ExternalOutput")

    # per-(layer, m-tile) pipelined AllGather staging: each core
    # contributes its 512-node h slice in node-major fp8; gather output
    # is [NC*512, H] ordered (core, subchunk j, partition p).
    ag_in = [[nc.dram_tensor(f"ag_in{l}_{m}", [512, H], FP8)
              for m in range(3)] for l in range(2)]
    ag_out = [[nc.dram_tensor(f"ag_out{l}_{m}", [NC * 512, H], FP8,
                              addr_space="Shared")
               for m in range(3)] for l in range(2)]
    # global-sum-pool exchange: AllGather of each core's [H,1] partial
    agp_in = nc.dram_tensor("agp_in", [H, 1], F32)
    agp_out = nc.dram_tensor("agp_out", [NC * H, 1], F32,
                             addr_space="Shared")
    rg = [list(range(NC))]

    with tile.TileContext(nc) as tc:
        with (
            tc.tile_pool(name="const", bufs=1) as constp,
            tc.tile_pool(name="atres", bufs=1) as atresp,
            tc.tile_pool(name="xstat", bufs=1) as xstatp,
            tc.tile_pool(name="hstat", bufs=2) as hstatp,
            tc.tile_pool(name="hT", bufs=2) as hTp,
            tc.tile_pool(name="hnat", bufs=2) as hnatp,
            tc.tile_pool(name="ep", bufs=2) as ep,
            tc.tile_pool(name="znp", bufs=1) as znp,
            tc.tile_pool(name="agg_ps", bufs=3, space=bass.MemorySpace.PSUM) as agg_ps,
            tc.tile_pool(name="z_ps", bufs=2, space=bass.MemorySpace.PSUM) as z_ps,
            tc.tile_pool(name="bc_ps", bufs=1, space=bass.MemorySpace.PSUM) as bc_ps,
            tc.tile_pool(name="t_ps", bufs=2, space=bass.MemorySpace.PSUM) as t_ps,
        ):
            # x-stationary first on gpsimd: host stores xs partition-major
            # ([128, KC*F]) so each load is one contiguous 6KB descriptor
            # per partition — two issues total.
            xs = xstatp.tile([128, KC, F], FP8)
            xs_r = xs_d.ap().rearrange("p (k f) -> p k f", f=F)
            nc.gpsimd.dma_start(xs[:, 0:48, :], xs_r[:, 0:48, :])
            nc.gpsimd.dma_start(xs[:, 48:96, :], xs_r[:, 48:96, :])

            # resident A^T shard, streamed in COLUMN-THIRDS matching the
            # three m-tiles: L1's m-tile 0 finishes right after the first
            # third lands, so its tail + AllGather overlap the remaining
            # two thirds of the stream (and absorb cross-core launch
            # skew). Host stores the shard partition-major third-major so
            # every 8-chunk DMA is 128 contiguous 4KB descriptors. All 36
            # stream DMAs ride the sync queue ALONE — DMA-ring
            # backpressure on a shared queue would block tail work.
            att = atresp.tile([128, 3, KC, 512], FP8)
            at_r = at_d.ap().rearrange("p (t k s) -> p t k s", t=3, s=512)
            for t in range(3):
                for g in range(12):
                    nc.sync.dma_start(
                        att[:, t, 8 * g:8 * g + 8, :],
                        at_r[:, t, 8 * g:8 * g + 8, :])

            # ---- constants
            def cload(dram, shape, dt=F32):
                t = constp.tile(shape, dt, tag=dram.name)
                nc.gpsimd.dma_start(t[:], dram[:, :])
                return t

            w1t = cload(w1t_d, [F, H], BF16)
            w1b = cload(w1b_d, [F, H], F32R)
            w2t = cload(w2t_d, [H, H], F32R)
            w2b = cload(w2b_d, [H, H], F32R)
            w3t = cload(w3t_d, [H, H], F32R)
            w3b = cload(w3b_d, [H, H], F32R)
            wf1p = cload(wf1p_d, [128, 2 * H])
            wf2 = cload(wf2_d, [2 * H, 1])
            b1 = cload(b1_d, [H, 1])
            b2 = cload(b2_d, [H, 1])
            b3 = cload(b3_d, [H, 1])
            bf1 = cload(bf1_d, [2 * H, 1])
            bf2 = cload(bf2_d, [1, 1])
            i32 = cload(i32_d, [32, 32], F32R)
            ones_m = cload(ones_d, [H, H], F32R)
            xt = cload(xt_d, [F, SH], BF16)
            rc = cload(rc_d, [F, SH], BF16)

            # ACT table prefetch targets: 1-elem dummy activations force
            # the ~1.3us table load to happen while other engines work.
            dmy = constp.tile([1, 1], F32, tag="dmy")

            def prefetch(func, _name):
                nc.scalar.activation(dmy[:, :], bf2[:1, :1], func)

            # preload the SQRT table during the L1 stream
            prefetch(SQRT, "pre_sqrt_init")

            def big_matmul(pagg, h_stat, m0, mw, j, start, stop):
                nc.tensor.matmul(
                    pagg[:, :mw], h_stat[:, 2 * j:2 * j + 2, :],
                    att[:, m0 // 512, 2 * j:2 * j + 2, :mw],
                    start=start, stop=stop, perf_mode=DR)

            def tail(li, fl, mi, m0, mw, pagg, hT_in, wtop, wbot, b,
                     hTn, hnat, red=None, last=False):
                """per-m-tile epilogue: mean-scale, dense, l2norm, tanh,
                then transpose the fp8 node-major h; m-tile 0 exports and
                triggers AG_a, m-tile 2 exports m-tiles 1+2 as AG_b.
                red: (tile, col) partial sum-pool column (fused into the
                tanh's accum_out)."""
                # mean-scale stays on DVE (gpsimd has no PSUM port)
                aggs = ep.tile([F, 512], F32R, tag="aggs")
                nc.vector.tensor_mul(
                    aggs[:fl, :mw], pagg[:fl, :mw], rc[:fl, m0:m0 + mw])
                pz = z_ps.tile([H, 512], F32, tag="pz")
                nc.tensor.matmul(pz[:, :mw], wtop[:, :], hT_in[:, m0:m0 + mw],
                                 start=True, stop=False)
                nc.tensor.matmul(pz[:, :mw], wbot[:, :], aggs[:fl, :mw],
                                 start=False, stop=True)
                zb = ep.tile([H, 512], F32, tag="zb")
                nc.vector.tensor_scalar_add(zb[:, :mw], pz[:, :mw], b[:])
                # row l2-norm over features (partition dim): sumsq via
                # ones-matmul broadcast back to H partitions.
                sq = ep.tile([H, 512], F32R, tag="sqzn")
                nc.vector.tensor_mul(sq[:, :mw], zb[:, :mw], zb[:, :mw])
                pbc = bc_ps.tile([H, 512], F32, tag="pbc")
                nc.tensor.matmul(pbc[:, :mw], ones_m[:, :], sq[:, :mw],
                                 start=True, stop=True)
                ssb = ep.tile([H, 512], F32, tag="ssb")
                nc.vector.tensor_scalar_max(ssb[:, :mw], pbc[:, :mw], 1e-12)
                srt = ep.tile([H, 512], F32, tag="sqzn")
                nc.scalar.sqrt(srt[:, :mw], ssb[:, :mw])
                if red is not None and mi > 0:
                    # pool layer (L3), m-tiles 1-2: DEFER the tanh so
                    # sqrt1/sqrt2 share SQRT table residency and the two
                    # tanhs share one TANH load at the layer end.
                    # recip+mul run eagerly on DVE.
                    rn = ep.tile([H, 512], F32, tag="rn")
                    nc.vector.reciprocal_approx_fast(rn[:, :mw],
                                                     srt[:, :mw])
                    znd = znp.tile([H, 512], F32, tag=f"znd{mi}")
                    nc.vector.tensor_mul(znd[:, :mw], zb[:, :mw],
                                         rn[:, :mw])
                    rtile, rcol = red

                    def fin():
                        nc.scalar.activation(
                            hTn[:, m0:m0 + mw], znd[:, :mw], TANH,
                            accum_out=rtile[:, rcol:rcol + 1])
                    return (fin, znd)
                # swap the table to TANH under the DVE recip+mul. The
                # dummy READS the sqrt's output so the scheduler cannot
                # hoist it ahead of the real sqrt.
                nc.scalar.activation(dmy[:, :], srt[0:1, 0:1], TANH)
                rn = ep.tile([H, 512], F32, tag="rn")
                nc.vector.reciprocal_approx_fast(rn[:, :mw], srt[:, :mw])
                zn = ep.tile([H, 512], F32, tag="sqzn")
                nc.vector.tensor_mul(zn[:, :mw], zb[:, :mw], rn[:, :mw])
                if red is not None:
                    rtile, rcol = red
                    nc.scalar.activation(hTn[:, m0:m0 + mw], zn[:, :mw],
                                         TANH,
                                         accum_out=rtile[:, rcol:rcol + 1])
                else:
                    nc.scalar.activation(hTn[:, m0:m0 + mw], zn[:, :mw],
                                         TANH)
                if not last:
                    # re-arm the SQRT table for the next tail, anchored
                    # behind the real tanh via a read of its output.
                    nc.scalar.activation(dmy[:, :], hTn[0:1, m0:m0 + 1],
                                         SQRT)
                if hnat is None:
                    return None

                def export(h_stat):
                    """transpose to node-major, export, AllGather, and
                    reload the gathered chunk region. DEFERRED by the
                    caller until after the next m-tile's matmuls are
                    emitted: the PE wait-queue holds only 4 blocked
                    instructions, so emitting the tanh-gated transposes
                    before ready matmuls head-of-line-blocks the PE."""
                    js = m0 // 128
                    for j in range(js, js + 4):
                        pt = t_ps.tile([128, H], F32R, tag="pt")
                        nc.tensor.transpose(
                            pt[:, :], hTn[:, j * 128:(j + 1) * 128],
                            i32[:, :])
                        nc.vector.tensor_copy(hnat[:, j, :], pt[:, :])
                    agr = ag_in[li][mi].ap().rearrange(
                        "(j p) f -> p j f", p=128)
                    nc.scalar.dma_start(agr[:, :, :], hnat[:, js:js + 4, :])
                    nc.gpsimd.collective_compute(
                        "AllGather", mybir.AluOpType.bypass,
                        replica_groups=rg,
                        ins=[ag_in[li][mi].ap().opt()],
                        outs=[ag_out[li][mi].ap().opt()])
                    if h_stat is not None:
                        load_hstat(li, h_stat, mi)
                return export

            def warm_chain(tag, n):
                """dummy matmuls to keep the PE HAM-warm across a gap."""
                if n <= 0:
                    return
                pw = bc_ps.tile([H, 512], F32, tag="pbc", name=f"warm{tag}")
                for d in range(n):
                    nc.tensor.matmul(pw[:, :], xs[:, 0, 0:H], xs[:, 0:4, :],
                                     start=(d == 0), stop=(d == n - 1))

            def load_hstat(li, h_stat, mi):
                """pull AG #mi's output into the stationary h tile: with
                the CHUNK_PERM layout these are chunks [32mi, 32mi+32)."""
                agor = ag_out[li][mi].ap().rearrange(
                    "(k p) f -> p k f", p=128)
                nc.sync.dma_start(h_stat[:, 32 * mi:32 * mi + 32, :],
                                  agor)

            # ----------------- layer 1: column-third-outer, k-inner;
            # m-tile t's tail + AllGather overlap thirds t+1.. of the
            # stream.
            hT1 = hTp.tile([H, SH], F32R, tag="hTn", name="hTn0")
            hnat0 = hnatp.tile([128, NJ, H], FP8, tag="hnat", name="hnat0")
            hs1 = hstatp.tile([128, KC, H], FP8, tag="hstat", name="hstat0")
            # get the PE HAM-warm (2.4 GHz) before/while the stream starts:
            # a cold PE (1.2 GHz) paces L1 slower than the DMA stream.
            warm_chain("pre", 18)
            pend = None
            for t, (m0, mw) in enumerate(MT):
                pagg = agg_ps.tile([F, 512], F32, tag="pagg",
                                   name=f"pagg0_{t}")
                for j in range(KP):
                    big_matmul(pagg, xs, m0, mw, j,
                               start=(j == 0), stop=(j == KP - 1))
                    if t == 0 and 0 < j < 8:
                        # pad early DMA-wait gaps so HAM's activity window
                        # sees a busy PE and un-throttles to full clock
                        warm_chain(f"pad{j}", 3)
                    if j == 3 and pend is not None:
                        # a 4-pair prefix of ready matmuls sits ahead of
                        # the previous tail's 4 tanh-gated transposes, so
                        # they fill the PE wait-queue without blocking
                        pend(hs1)
                        pend = None
                pend = tail(0, F, t, m0, mw, pagg, xt, w1t, w1b, b1,
                            hT1, hnat0)
            warm_chain("sp0", 4)
            pend(hs1)
            warm_chain("b0", 2 * WARM)

            # ----------------- layers 2-3: m-outer, A + h resident in SBUF
            def layer23(li, hs, hT_in, wtop, wbot, b, hnat, pool=None,
                        hs_next=None):
                hTn = hTp.tile([H, SH], F32R, tag="hTn", name=f"hTn{li}")
                pend = None
                fins = []
                for mi, (m0, mw) in enumerate(MT):
                    pagg = agg_ps.tile([H, 512], F32, tag="pagg",
                                       name=f"pagg{li}_{mi}")
                    for j in range(KP):
                        big_matmul(pagg, hs, m0, mw, j,
                                   start=(j == 0), stop=(j == KP - 1))
                        if j == 3 and pend is not None:
                            pend(hs_next)
                            pend = None
                    if mi == 2 and hnat is not None:
                        warm_chain(f"l{li}", WARM)
                    r = tail(li, H, mi, m0, mw, pagg, hT_in, wtop,
                             wbot, b, hTn, hnat,
                             red=(pool, mi) if pool is not None else None,
                             last=(pool is not None and mi == 2))
                    if pool is not None:
                        if r is not None:
                            fins.append(r)
                    else:
                        pend = r
                if pend is not None:
                    warm_chain(f"sp{li}", 4)
                    pend(hs_next)
                if fins:
                    # one TANH table load, anchored behind the last
                    # m-tile's recip/mul chain, then the deferred tanhs
                    nc.scalar.activation(dmy[:, :], fins[-1][1][0:1, 0:1],
                                         TANH)
                    for fin, _ in fins:
                        fin()
                return hTn

            hnat1 = hnatp.tile([128, NJ, H], FP8, tag="hnat", name="hnat1")
            hs2 = hstatp.tile([128, KC, H], FP8, tag="hstat", name="hstat1")
            hT2 = layer23(1, hs1, hT1, w2t, w2b, b2, hnat1, hs_next=hs2)
            warm_chain("b1", 2 * WARM)
            pool4 = ep.tile([H, 4], F32, tag="pT")
            hT3 = layer23(2, hs2, hT2, w3t, w3b, b3, None, pool=pool4)

            # combine the per-m-tile pool partials (padded nodes are 0),
            # AllGather the [H,1] partials, and reduce 8 blocks on-core:
            # [256,1] -> [128,2] -> DVE free-sum -> [128,1] -> i128 matmul.
            pT = ep.tile([H, 1], F32, tag="pS")
            nc.vector.reduce_sum(pT[:, :], pool4[:, 0:3],
                                 axis=mybir.AxisListType.X)
            nc.scalar.dma_start(agp_in[:, :], pT[:])
            nc.gpsimd.collective_compute(
                "AllGather", mybir.AluOpType.bypass, replica_groups=rg,
                ins=[agp_in.ap().opt()], outs=[agp_out.ap().opt()])
            pG = ep.tile([128, 2], F32, tag="pG")
            nc.gpsimd.dma_start(
                pG[:, :], agp_out.ap().rearrange("(a p) c -> p (a c)", a=2))
            pH = ep.tile([128, 1], F32, tag="pH")
            nc.vector.reduce_sum(pH[:, :], pG[:, :],
                                 axis=mybir.AxisListType.X)
            # final MLP (redundant on every core); wf1p = tile(Wf1, (4,1))
            # host-folds the 4-block pool reduction into the dense layer
            pq = z_ps.tile([2 * H, 1], F32, tag="pz")
            nc.tensor.matmul(pq[:, :], wf1p[:, :], pH[:, :], start=True,
                             stop=True)
            q = ep.tile([2 * H, 1], F32, tag="q")
            nc.scalar.activation(q[:, :], pq[:, :], TANH, bias=bf1[:])
            po = z_ps.tile([1, 1], F32, tag="pz")
            nc.tensor.matmul(po[:, :], wf2[:, :], q[:, :], start=True,
                             stop=True)
            ob = ep.tile([1, 1], F32, tag="ob")
            nc.vector.tensor_scalar_add(ob[:, :], po[:, :], bf2[:])
            nc.gpsimd.dma_start(out_d[:, :], ob[:])

    nc.compile()
    return nc


# ---------------------------------------------------------------- host prep
def _prep(inputs):
    x = np.asarray(inputs["x"], np.float32)
    a = np.asarray(inputs["a"], np.float32)
    diag = np.diagonal(a).copy()
    add = (np.abs(diag) < TOL).astype(np.float32)
    deg = a.sum(axis=1) + add          # row sums of a_hat
    recip = np.ones(NP, np.float32)
    recip[:N] = 1.0 / deg

    # row-block permutation implementing CHUNK_PERM (see top of file)
    row_perm = (np.asarray(CHUNK_PERM)[:, None] * 128
                + np.arange(128)[None, :]).reshape(-1)

    x_pad = np.zeros((NP, F), np.float32)
    x_pad[:N] = x
    # partition-major xs: [128, KC*F], row p = concat_k x[chunk k, p]
    xs = (x_pad[row_perm].reshape(KC, 128, F).transpose(1, 0, 2)
          .reshape(128, KC * F).astype(NP_FP8))

    w1 = np.asarray(inputs["W1"], np.float32)
    common = {
        "xs": xs,
        "w1t": w1[:F].astype(NP_BF16), "w1b": w1[F:].copy(),
        "w2t": np.asarray(inputs["W2"], np.float32)[:H].copy(),
        "w2b": np.asarray(inputs["W2"], np.float32)[H:].copy(),
        "w3t": np.asarray(inputs["W3"], np.float32)[:H].copy(),
        "w3b": np.asarray(inputs["W3"], np.float32)[H:].copy(),
        "wf1p": np.tile(np.asarray(inputs["Wf1"], np.float32), (4, 1)),
        "wf2": np.asarray(inputs["Wf2"], np.float32),
        "b1": np.asarray(inputs["b1"], np.float32).reshape(H, 1),
        "b2": np.asarray(inputs["b2"], np.float32).reshape(H, 1),
        "b3": np.asarray(inputs["b3"], np.float32).reshape(H, 1),
        "bf1": np.asarray(inputs["bf1"], np.float32).reshape(2 * H, 1),
        "bf2": np.asarray(inputs["bf2"], np.float32).reshape(1, 1),
        "i32": np.eye(32, dtype=np.float32),
        "ones": np.ones((H, H), dtype=np.float32),
    }

    in_maps = []
    for c in range(NC):
        r0 = c * SH
        r1 = min((c + 1) * SH, N)
        nrow = max(r1 - r0, 0)
        at = np.zeros((NP, SH), NP_FP8)
        if nrow > 0:
            blk = a[r0:r1].T.astype(NP_FP8)         # [N(12000), nrow]
            at[:N, :nrow] = blk
            # self-loops on approximately-zero diagonal entries
            idx = np.arange(nrow)
            gi = r0 + idx
            sel = add[gi] > 0
            at[gi[sel], idx[sel]] = np.asarray(
                a[gi[sel], gi[sel]] + 1.0, NP_FP8)
        # partition-major third-major: row p = concat over (third t,
        # chunk k') of at[128k'+p, 512t:512t+512]
        at = (at[row_perm].reshape(KC, 128, 3, 512)
              .transpose(1, 2, 0, 3).reshape(128, 3 * KC * 512))
        xt = np.zeros((F, SH), NP_BF16)
        if nrow > 0:
            xt[:, :nrow] = x[r0:r1].T.astype(NP_BF16)
        rcb = np.broadcast_to(recip[r0:r0 + SH].astype(NP_BF16),
                              (F, SH)).copy()
        m = dict(common)
        m.update({"at": at, "xt": xt, "rc": rcb})
        in_maps.append(m)
    return in_maps


# -------------------------------------------------------------------- kernel
def kernel(**inputs):
    global LAST_EXEC_NS
    if "nc" not in _CACHE:
        _CACHE["nc"] = _build()
    nc = _CACHE["nc"]
    in_maps = _prep(inputs)
    res = run_bass_kernel_spmd(nc, in_maps, core_ids=list(range(NC)))
    LAST_EXEC_NS = res.exec_time_ns
    return np.asarray(res.results[0]["out"], np.float32).reshape(1, 1)
